# revision 24
# baseline (speedup 1.0000x reference)
"""Trainium2 Bass kernel for nn_MultiHeadAttention (B=2, S=2048, D=1024, H=16, causal).

Strategy (tensor-parallel over heads, per the sharding hint):
  - Each of the 8 cores computes H/8 = 2 heads end-to-end:
      QKV projections for its heads, causal flash-style attention
      (exp without max-subtraction -- scores are ~N(0,1) so exp never
      overflows), partial output projection against its w_o row-slice.
  - The final all-reduce after w_o (see sharding hint) is realized in the
    unshard step: each core returns a bf16 partial [T, D]; the host sums the
    8 partials in fp32.  Zero on-device collectives.
  - All inputs/weights are host-cast to bf16 and pre-arranged so every DMA
    is partition-contiguous (128 descriptors of 2-8KB) -- small-descriptor
    DGE stalls dominated the v1 startup.  q/k/v ride three different DMA
    queues (sync/gpsimd/scalar).  Causal structure is exploited by skipping
    fully-masked 128-wide key blocks.
  - Engine roles: PE matmuls; ACT = exp + per-chunk psum evacuation only
    (exp cadence paces the attention inner loop); DVE = casts/normalize;
    Pool = k-loads, stores, partition broadcasts.  Output projection quanta
    are spread through later chunks as PE filler.

Self-contained: hardcodes shapes; no sibling imports.
"""

import sys

if "/opt/trn_rl_repo" not in sys.path:
    sys.path.insert(0, "/opt/trn_rl_repo")

import numpy as np

import concourse.bass as bass
import concourse.mybir as mybir
import concourse.tile as tile
from concourse import bacc
from concourse.bass_utils import run_bass_kernel_spmd

B, S, D, H = 2, 2048, 1024, 16
DK = D // H          # 64 head dim
N_CORES = 8
HPC = H // N_CORES   # 2 heads per core
DPC = DK * HPC       # 128 local feature columns per core
T = B * S            # 4096 tokens
NT = T // 128        # 32 token blocks of 128
NC = S // 512        # 4 query chunks of 512 per batch
SCALE = 1.0 / np.sqrt(np.float32(DK))

f32 = mybir.dt.float32
bf16 = mybir.dt.bfloat16

_CACHED = {}


def build_nc():
    nc = bacc.Bacc("TRN2", target_bir_lowering=False, debug=False, num_devices=N_CORES)

    # x tensors: row-block r = 128*(2*tp+half)+partition, cols = 8 feat-blocks * 512 tok
    qT = nc.dram_tensor("qT", [1024, 4096], bf16, kind="ExternalInput")
    kT = nc.dram_tensor("kT", [1024, 4096], bf16, kind="ExternalInput")
    vT = nc.dram_tensor("vT", [1024, 4096], bf16, kind="ExternalInput")
    # weights pre-arranged partition-major: [128, 8 feat-blocks * 128]
    wqT = nc.dram_tensor("wqT", [128, 1024], bf16, kind="ExternalInput")
    wkT = nc.dram_tensor("wkT", [128, 1024], bf16, kind="ExternalInput")
    wvT = nc.dram_tensor("wvT", [128, 1024], bf16, kind="ExternalInput")
    woT = nc.dram_tensor("woT", [128, 1024], bf16, kind="ExternalInput")
    msk = nc.dram_tensor("msk", [128, 128], bf16, kind="ExternalInput")
    # output: row = 128*group+partition, cols = 4 tok-blocks * 1024 feat
    outp = nc.dram_tensor("outp", [1024, 4096], bf16, kind="ExternalOutput")

    Exp = mybir.ActivationFunctionType.Exp
    MUL = mybir.AluOpType.mult

    with tile.TileContext(nc) as tc:
        with (
            tc.tile_pool(name="res", bufs=1) as res,          # resident SBUF
            tc.tile_pool(name="xq", bufs=2) as xq_pool,       # q token slabs
            tc.tile_pool(name="xk", bufs=2) as xk_pool,       # k token slabs
            tc.tile_pool(name="xv", bufs=2) as xv_pool,       # v token slabs
            tc.tile_pool(name="ex", bufs=5) as ex_pool,       # exp tiles
            tc.tile_pool(name="rw", bufs=4) as rw_pool,       # ps_ctx evacuation
            tc.tile_pool(name="dv", bufs=2) as dv_pool,       # recip/bcast
            tc.tile_pool(name="ob", bufs=2) as ob_pool,       # ph3 output staging
            tc.tile_pool(name="p1", bufs=2, space="PSUM") as p1,      # proj + o-proj: 2 banks
            tc.tile_pool(name="psc", bufs=2, space="PSUM") as psc,    # scores: 4 banks
            tc.tile_pool(name="pcx", bufs=2, space="PSUM") as pcx,    # ctx accumulators: 2 banks
        ):
            # ---------------- prelude ----------------
            # each projection weight leads its own input stream's queue:
            # wq -> sync (ahead of q pieces), wk -> gpsimd (ahead of k),
            # wv -> scalar (ahead of v); wo/msk follow on gpsimd later.
            wq_sb = res.tile([128, 8, 128], bf16, tag="wq")
            nc.sync.dma_start(out=wq_sb[:], in_=wqT[:])

            filler = []

            def emit_filler(k=1):
                for _ in range(k):
                    if filler:
                        filler.pop(0)()

            def ph1_loads(tp):
                """Slab DMAs for token chunks 2tp, 2tp+1; 3 queues, 2 pieces each."""
                qt = xq_pool.tile([128, 2, 8, 512], bf16, tag="xq", name="qt")
                kt = xk_pool.tile([128, 2, 8, 512], bf16, tag="xk", name="kt")
                vt = xv_pool.tile([128, 2, 8, 512], bf16, tag="xv", name="vt")
                for half in range(2):
                    rows = slice(128 * (2 * tp + half), 128 * (2 * tp + half + 1))
                    nc.sync.dma_start(out=qt[:, half], in_=qT[rows, :])
                    nc.gpsimd.dma_start(out=kt[:, half], in_=kT[rows, :])
                    nc.scalar.dma_start(out=vt[:, half], in_=vT[rows, :])
                return qt, kt, vt

            wk_sb = res.tile([128, 8, 128], bf16, tag="wk")
            nc.gpsimd.dma_start(out=wk_sb[:], in_=wkT[:])
            wv_sb = res.tile([128, 8, 128], bf16, tag="wv")
            nc.scalar.dma_start(out=wv_sb[:], in_=wvT[:])

            tiles0 = ph1_loads(0)

            wo_sb = res.tile([128, 1024], bf16, tag="wo")
            nc.gpsimd.dma_start(out=wo_sb[:], in_=woT[:])
            mk_sb = res.tile([128, 128], bf16, tag="mk")
            nc.gpsimd.dma_start(out=mk_sb[:], in_=msk[:])

            # resident activations
            QHT = res.tile([128, T], bf16, tag="QHT")    # [d_local, t]
            KHT = res.tile([128, T], bf16, tag="KHT")
            V_sb = res.tile([128, NT * 130], bf16, tag="V")   # per t-block: 2 heads x (64 + ones)
            CTX = res.tile([128, T], bf16, tag="CTX")    # [d_local, t] post-softmax context

            nc.vector.memset(
                V_sb[:].rearrange("p (n x) -> p n x", x=65)[:, :, 64:65], 1.0
            )

            def ph1_quanta(tp, qt, kt, vt):
                """Queue the projection matmul groups for this tcpair as PE filler."""
                out = []
                for half in range(2):
                    tcn = 2 * tp + half
                    cols = slice(512 * tcn, 512 * (tcn + 1))

                    def q_quant(cols=cols, half=half):
                        ps_q = p1.tile([128, 512], f32, tag="p1", name="ps_q")
                        for kb in range(8):
                            nc.tensor.matmul(ps_q[:], wq_sb[:, kb, :], qt[:, half, kb, :], start=kb == 0, stop=kb == 7)
                        nc.vector.tensor_copy(QHT[:, cols], ps_q[:])

                    def k_quant(cols=cols, half=half):
                        ps_k = p1.tile([128, 512], f32, tag="p1", name="ps_k")
                        for kb in range(8):
                            nc.tensor.matmul(ps_k[:], wk_sb[:, kb, :], kt[:, half, kb, :], start=kb == 0, stop=kb == 7)
                        nc.vector.tensor_copy(KHT[:, cols], ps_k[:])

                    def v_quant(tcn=tcn, half=half):
                        ps_v = p1.tile([128, 512], f32, tag="p1", name="ps_v")
                        for i in range(4):
                            for kb in range(8):
                                nc.tensor.matmul(
                                    ps_v[:, 128 * i:128 * (i + 1)],
                                    vt[:, half, kb, 128 * i:128 * (i + 1)],
                                    wv_sb[:, kb, :],
                                    start=(kb == 0), stop=(kb == 7),
                                )
                        for i in range(4):
                            g = 4 * tcn + i
                            nc.vector.tensor_copy(
                                V_sb[:, 130 * g:130 * (g + 1)].rearrange("p (h x) -> p h x", x=65)[:, :, 0:64],
                                ps_v[:, 128 * i:128 * (i + 1)].rearrange("p (h x) -> p h x", x=64),
                            )

                    out += [q_quant, k_quant, v_quant]
                return out

            def ph2_chunk(b, c):
                """Causal attention for both heads, batch b, query chunk c (512 q)."""
                qcols = slice(2048 * b + 512 * c, 2048 * b + 512 * (c + 1))
                ps_ctx = {}
                for h in range(2):
                    ps_ctx[h] = pcx.tile([65, 512], f32, tag="ctx", name="ps_ctx")
                nblk = 4 * c + 4
                pend = None  # (j, band, ex) awaiting its ctx matmuls

                def emit_ctx(p):
                    j, band, ex = p
                    g = 16 * b + j
                    for h in range(2):
                        nc.tensor.matmul(
                            ps_ctx[h][:, band],
                            V_sb[:, 130 * g + 65 * h:130 * g + 65 * (h + 1)],
                            ex[:, 512 * h + band.start:512 * h + band.stop],
                            start=(j == 0), stop=(j == nblk - 1),
                        )

                for j in range(nblk):
                    kcols = slice(2048 * b + 128 * j, 2048 * b + 128 * (j + 1))
                    d = j - 4 * c
                    band = slice(128 * d, 512) if d > 0 else slice(0, 512)
                    qb = slice(qcols.start + band.start, qcols.stop)
                    sc = psc.tile([128, 1024], f32, tag="sc", name="sc")
                    for h in range(2):
                        rows = slice(64 * h, 64 * (h + 1))
                        nc.tensor.matmul(
                            sc[:, 512 * h + band.start:512 * h + band.stop],
                            KHT[rows, kcols], QHT[rows, qb], start=True, stop=True)
                    ex = ex_pool.tile([128, 1024], bf16, tag="ex", name="ex")
                    if band.start == 0:
                        nc.scalar.activation(ex[:], sc[:], Exp, scale=float(SCALE))
                    else:
                        sc3 = sc[:].rearrange("p (h q) -> p h q", h=2)[:, :, band.start:band.stop]
                        ex3 = ex[:].rearrange("p (h q) -> p h q", h=2)[:, :, band.start:band.stop]
                        nc.scalar.activation(ex3, sc3, Exp, scale=float(SCALE))
                    if d >= 0:
                        mband = slice(128 * d, 128 * (d + 1))
                        exm = ex[:].rearrange("p (h q) -> p h q", h=2)[:, :, mband]
                        mk3 = mk_sb[:].rearrange("p (h q) -> p h q", h=1).broadcast_to((128, 2, 128))
                        nc.vector.tensor_tensor(exm, exm, mk3, MUL)
                    if pend is not None:
                        # ration filler: early chunks take little so the
                        # exp-paced tails of long chunks stay fed
                        if c == 3:
                            emit_filler(2)
                        elif c == 2:
                            emit_filler(1)
                        elif j % 2 == 1:
                            emit_filler(1)
                        emit_ctx(pend)
                    pend = (j, band, ex)
                emit_ctx(pend)
                emit_filler(1)
                for h in range(2):
                    rows = slice(64 * h, 64 * (h + 1))
                    # evacuate the psum bank via ACT (idle at chunk ends) so the
                    # next chunk's ctx accumulation can reuse it immediately;
                    # 64p + 1p copies (a single 65p ACT copy is broken on HW)
                    raw = rw_pool.tile([64, 512], f32, tag="raw", name="raw")
                    nc.scalar.copy(raw[:], ps_ctx[h][0:64, :])
                    den = dv_pool.tile([1, 512], f32, tag="den")
                    nc.scalar.copy(den[:], ps_ctx[h][64:65, :])
                    rec = dv_pool.tile([1, 512], f32, tag="rec")
                    nc.vector.reciprocal_approx_fast(out=rec[:], in_=den[:])
                    bc = dv_pool.tile([64, 512], f32, tag="bc")
                    nc.gpsimd.partition_broadcast(bc[:], rec[:])
                    nc.vector.tensor_tensor(CTX[rows, qcols], raw[:], bc[:], MUL)
                # this chunk's output projection becomes PE filler for later chunks
                obholder = {}
                for tb in range(16 * b + 4 * c, 16 * b + 4 * (c + 1)):
                    def ph3_quant(tb=tb, obholder=obholder):
                        grp, idx = divmod(tb, 4)
                        if idx == 0:
                            obholder["ob"] = ob_pool.tile([128, 4, 1024], bf16, tag="ob", name="ob")
                        ob = obholder["ob"]
                        for e in range(2):
                            po = p1.tile([128, 512], f32, tag="p1", name="po")
                            nc.tensor.matmul(
                                po[:],
                                CTX[:, 128 * tb:128 * (tb + 1)],
                                wo_sb[:, 512 * e:512 * (e + 1)],
                                start=True, stop=True,
                            )
                            nc.vector.tensor_copy(ob[:, idx, 512 * e:512 * (e + 1)], po[:])
                        if idx == 3:
                            nc.gpsimd.dma_start(
                                out=outp[128 * grp:128 * (grp + 1), :], in_=ob[:])
                    filler.append(ph3_quant)

            # ---- schedule: loads run one tcpair ahead; projection matmuls and
            # ---- output-projection blocks fill PE gaps inside attention chunks
            for qf in ph1_quanta(0, *tiles0):
                qf()
            chunk_of_tp = {0: (0, 0, 1), 1: (0, 2, 3), 2: (1, 0, 1), 3: (1, 2, 3)}
            for tp in range(1, 4):
                tiles = ph1_loads(tp)
                filler.extend(ph1_quanta(tp, *tiles))
                b, ca, cb = chunk_of_tp[tp - 1]
                ph2_chunk(b, ca)
                ph2_chunk(b, cb)
            b, ca, cb = chunk_of_tp[3]
            ph2_chunk(b, ca)
            ph2_chunk(b, cb)
            emit_filler(len(filler))

    nc.compile()
    return nc


def _host_inputs(q, k, v, mask, w_q, w_k, w_v, w_o):
    import ml_dtypes

    nbf = ml_dtypes.bfloat16

    def arrange_x(x):
        # [T, D] tokens-major -> [slab, piece, partition, featblock*token]
        x2 = np.asarray(x, dtype=np.float32).reshape(T, D).T.astype(nbf)  # [D, T]
        x4 = x2.reshape(8, 128, 4, 2, 512)          # (a, p, tp, half, t)
        return np.ascontiguousarray(x4.transpose(2, 3, 1, 0, 4).reshape(1024, 4096))

    def arrange_w(wT):
        # wT [D, DPC] -> [128, 8*128] partition-major image of the SBUF tile
        w4 = wT.astype(nbf).reshape(8, 128, 128)    # (a, p, d)
        return np.ascontiguousarray(w4.transpose(1, 0, 2).reshape(128, 1024))

    w_q = np.asarray(w_q, dtype=np.float32)
    w_k = np.asarray(w_k, dtype=np.float32)
    w_v = np.asarray(w_v, dtype=np.float32)
    w_o = np.asarray(w_o, dtype=np.float32)
    mask2d = np.asarray(mask).reshape(S, S)

    q4, k4, v4 = arrange_x(q), arrange_x(k), arrange_x(v)
    # single 128x128 tril mask for the mixed band of every diagonal block:
    # valid(r, u) = mask2d[u, r] on the leading 128x128 (= u >= r for causal)
    mk = np.ascontiguousarray(mask2d[0:128, 0:128].T.astype(nbf))

    in_maps = []
    for m in range(N_CORES):
        sl = slice(DPC * m, DPC * (m + 1))
        in_maps.append({
            "qT": q4,
            "kT": k4,
            "vT": v4,
            "wqT": arrange_w(w_q[sl, :].T),
            "wkT": arrange_w(w_k[sl, :].T),
            "wvT": arrange_w(w_v[sl, :].T),
            "woT": np.ascontiguousarray(w_o[:, sl].T.astype(nbf)),
            "msk": mk,
        })
    return in_maps


def kernel(q, k, v, mask, w_q, w_k, w_v, w_o, _trace=False, _results=None):
    in_maps = _host_inputs(q, k, v, mask, w_q, w_k, w_v, w_o)
    if "nc" not in _CACHED:
        _CACHED["nc"] = build_nc()
    nc = _CACHED["nc"]
    res = run_bass_kernel_spmd(
        nc, in_maps, core_ids=list(range(N_CORES)), trace=_trace
    )
    if _results is not None:
        _results.append(res)
    out = np.zeros((1024, 4096), dtype=np.float32)
    for m in range(N_CORES):
        out += np.asarray(res.results[m]["outp"], dtype=np.float32)
    # [group, partition, tokblock*feat] -> [T, D]
    out = out.reshape(8, 128, 4, 1024).transpose(0, 2, 1, 3).reshape(T, D)
    return out.reshape(B, S, D)


# revision 28
# speedup vs baseline: 1.0544x; 1.0544x over previous
"""Trainium2 Bass kernel for nn_MultiHeadAttention (B=2, S=2048, D=1024, H=16, causal).

Strategy (tensor-parallel over heads, per the sharding hint):
  - Each of the 8 cores computes H/8 = 2 heads end-to-end:
      QKV projections for its heads, causal flash-style attention
      (exp without max-subtraction -- scores are ~N(0,1) so exp never
      overflows), partial output projection against its w_o row-slice.
  - The final all-reduce after w_o (see sharding hint) is realized in the
    unshard step: each core returns a bf16 partial [T, D]; the host sums the
    8 partials in fp32.  Zero on-device collectives.
  - All inputs/weights are host-cast to bf16 and pre-arranged so every DMA
    is partition-contiguous (128 descriptors of 2-8KB) -- small-descriptor
    DGE stalls dominated the v1 startup.  q/k/v ride three different DMA
    queues (sync/gpsimd/scalar).  Causal structure is exploited by skipping
    fully-masked 128-wide key blocks.
  - Engine roles: PE matmuls; ACT = exp + per-chunk psum evacuation only
    (exp cadence paces the attention inner loop); DVE = casts/normalize;
    Pool = k-loads, stores, partition broadcasts.  Output projection quanta
    are spread through later chunks as PE filler.

Self-contained: hardcodes shapes; no sibling imports.
"""

import sys

if "/opt/trn_rl_repo" not in sys.path:
    sys.path.insert(0, "/opt/trn_rl_repo")

import numpy as np

import concourse.bass as bass
import concourse.mybir as mybir
import concourse.tile as tile
from concourse import bacc
from concourse.bass_utils import run_bass_kernel_spmd

B, S, D, H = 2, 2048, 1024, 16
DK = D // H          # 64 head dim
N_CORES = 8
HPC = H // N_CORES   # 2 heads per core
DPC = DK * HPC       # 128 local feature columns per core
T = B * S            # 4096 tokens
NT = T // 128        # 32 token blocks of 128
NC = S // 512        # 4 query chunks of 512 per batch
SCALE = 1.0 / np.sqrt(np.float32(DK))

f32 = mybir.dt.float32
bf16 = mybir.dt.bfloat16

_CACHED = {}


def build_nc():
    nc = bacc.Bacc("TRN2", target_bir_lowering=False, debug=False, num_devices=N_CORES)

    # x tensors: row-block r = 128*(2*tp+half)+partition, cols = 8 feat-blocks * 512 tok
    qT = nc.dram_tensor("qT", [1024, 4096], bf16, kind="ExternalInput")
    kT = nc.dram_tensor("kT", [1024, 4096], bf16, kind="ExternalInput")
    vT = nc.dram_tensor("vT", [1024, 4096], bf16, kind="ExternalInput")
    # weights pre-arranged partition-major: [128, 8 feat-blocks * 128]
    wqT = nc.dram_tensor("wqT", [128, 1024], bf16, kind="ExternalInput")
    wkT = nc.dram_tensor("wkT", [128, 1024], bf16, kind="ExternalInput")
    wvT = nc.dram_tensor("wvT", [128, 1024], bf16, kind="ExternalInput")
    woT = nc.dram_tensor("woT", [128, 1024], bf16, kind="ExternalInput")
    msk = nc.dram_tensor("msk", [128, 128], bf16, kind="ExternalInput")
    # output: row = 128*group+partition, cols = 4 tok-blocks * 1024 feat
    outp = nc.dram_tensor("outp", [1024, 4096], bf16, kind="ExternalOutput")

    Exp = mybir.ActivationFunctionType.Exp
    MUL = mybir.AluOpType.mult

    with tile.TileContext(nc) as tc:
        with (
            tc.tile_pool(name="res", bufs=1) as res,          # resident SBUF
            tc.tile_pool(name="xq", bufs=2) as xq_pool,       # q token slabs
            tc.tile_pool(name="xk", bufs=2) as xk_pool,       # k token slabs
            tc.tile_pool(name="xv", bufs=2) as xv_pool,       # v token slabs
            tc.tile_pool(name="ex", bufs=6) as ex_pool,       # exp tiles
            tc.tile_pool(name="rw", bufs=4) as rw_pool,       # ps_ctx evacuation
            tc.tile_pool(name="dv", bufs=2) as dv_pool,       # recip/bcast
            tc.tile_pool(name="ob", bufs=2) as ob_pool,       # ph3 output staging
            tc.tile_pool(name="p1", bufs=2, space="PSUM") as p1,      # proj + o-proj: 2 banks
            tc.tile_pool(name="psc", bufs=2, space="PSUM") as psc,    # scores: 4 banks
            tc.tile_pool(name="pcx", bufs=2, space="PSUM") as pcx,    # ctx accumulators: 2 banks
        ):
            # ---------------- prelude ----------------
            # each projection weight leads its own input stream's queue:
            # wq -> sync (ahead of q pieces), wk -> gpsimd (ahead of k),
            # wv -> scalar (ahead of v); wo/msk follow on gpsimd later.
            wq_sb = res.tile([128, 8, 128], bf16, tag="wq")
            nc.sync.dma_start(out=wq_sb[:], in_=wqT[:])

            filler = []

            def emit_filler(k=1):
                for _ in range(k):
                    if filler:
                        filler.pop(0)()

            def ph1_loads(tp):
                """Slab DMAs for token chunks 2tp, 2tp+1; 3 queues, 2 pieces each."""
                qt = xq_pool.tile([128, 2, 8, 512], bf16, tag="xq", name="qt")
                kt = xk_pool.tile([128, 2, 8, 512], bf16, tag="xk", name="kt")
                vt = xv_pool.tile([128, 2, 8, 512], bf16, tag="xv", name="vt")
                for half in range(2):
                    rows = slice(128 * (2 * tp + half), 128 * (2 * tp + half + 1))
                    nc.sync.dma_start(out=qt[:, half], in_=qT[rows, :])
                    nc.gpsimd.dma_start(out=kt[:, half], in_=kT[rows, :])
                    nc.scalar.dma_start(out=vt[:, half], in_=vT[rows, :])
                return qt, kt, vt

            wk_sb = res.tile([128, 8, 128], bf16, tag="wk")
            nc.gpsimd.dma_start(out=wk_sb[:], in_=wkT[:])
            wv_sb = res.tile([128, 8, 128], bf16, tag="wv")
            nc.scalar.dma_start(out=wv_sb[:], in_=wvT[:])

            tiles0 = ph1_loads(0)

            wo_sb = res.tile([128, 1024], bf16, tag="wo")
            nc.gpsimd.dma_start(out=wo_sb[:], in_=woT[:])
            mk_sb = res.tile([128, 128], bf16, tag="mk")
            nc.gpsimd.dma_start(out=mk_sb[:], in_=msk[:])

            # resident activations
            QHT = res.tile([128, T], bf16, tag="QHT")    # [d_local, t]
            KHT = res.tile([128, T], bf16, tag="KHT")
            V_sb = res.tile([128, NT * 130], bf16, tag="V")   # per t-block: 2 heads x (64 + ones)
            CTX = res.tile([128, T], bf16, tag="CTX")    # [d_local, t] post-softmax context

            nc.vector.memset(
                V_sb[:].rearrange("p (n x) -> p n x", x=65)[:, :, 64:65], 1.0
            )

            def ph1_quanta(tp, qt, kt, vt):
                """Queue the projection matmul groups for this tcpair as PE filler."""
                out = []
                for half in range(2):
                    tcn = 2 * tp + half
                    cols = slice(512 * tcn, 512 * (tcn + 1))

                    def q_quant(cols=cols, half=half):
                        ps_q = p1.tile([128, 512], f32, tag="p1", name="ps_q")
                        for kb in range(8):
                            nc.tensor.matmul(ps_q[:], wq_sb[:, kb, :], qt[:, half, kb, :], start=kb == 0, stop=kb == 7)
                        nc.vector.tensor_copy(QHT[:, cols], ps_q[:])

                    def k_quant(cols=cols, half=half):
                        ps_k = p1.tile([128, 512], f32, tag="p1", name="ps_k")
                        for kb in range(8):
                            nc.tensor.matmul(ps_k[:], wk_sb[:, kb, :], kt[:, half, kb, :], start=kb == 0, stop=kb == 7)
                        nc.vector.tensor_copy(KHT[:, cols], ps_k[:])

                    def v_quant(tcn=tcn, half=half):
                        ps_v = p1.tile([128, 512], f32, tag="p1", name="ps_v")
                        for i in range(4):
                            for kb in range(8):
                                nc.tensor.matmul(
                                    ps_v[:, 128 * i:128 * (i + 1)],
                                    vt[:, half, kb, 128 * i:128 * (i + 1)],
                                    wv_sb[:, kb, :],
                                    start=(kb == 0), stop=(kb == 7),
                                )
                        for i in range(4):
                            g = 4 * tcn + i
                            nc.vector.tensor_copy(
                                V_sb[:, 130 * g:130 * (g + 1)].rearrange("p (h x) -> p h x", x=65)[:, :, 0:64],
                                ps_v[:, 128 * i:128 * (i + 1)].rearrange("p (h x) -> p h x", x=64),
                            )

                    out += [q_quant, k_quant, v_quant]
                return out

            def ph2_chunk(b, c):
                """Causal attention for both heads, batch b, query chunk c (512 q)."""
                qcols = slice(2048 * b + 512 * c, 2048 * b + 512 * (c + 1))
                ps_ctx = {}
                for h in range(2):
                    ps_ctx[h] = pcx.tile([65, 512], f32, tag="ctx", name="ps_ctx")
                nblk = 4 * c + 4
                pend = None  # (j, band, ex) awaiting its ctx matmuls

                def emit_ctx(p):
                    j, band, ex = p
                    g = 16 * b + j
                    for h in range(2):
                        nc.tensor.matmul(
                            ps_ctx[h][:, band],
                            V_sb[:, 130 * g + 65 * h:130 * g + 65 * (h + 1)],
                            ex[:, 512 * h + band.start:512 * h + band.stop],
                            start=(j == 0), stop=(j == nblk - 1),
                        )

                for j in range(nblk):
                    kcols = slice(2048 * b + 128 * j, 2048 * b + 128 * (j + 1))
                    d = j - 4 * c
                    band = slice(128 * d, 512) if d > 0 else slice(0, 512)
                    qb = slice(qcols.start + band.start, qcols.stop)
                    sc = psc.tile([128, 1024], f32, tag="sc", name="sc")
                    for h in range(2):
                        rows = slice(64 * h, 64 * (h + 1))
                        nc.tensor.matmul(
                            sc[:, 512 * h + band.start:512 * h + band.stop],
                            KHT[rows, kcols], QHT[rows, qb], start=True, stop=True)
                    ex = ex_pool.tile([128, 1024], bf16, tag="ex", name="ex")
                    if band.start == 0:
                        nc.scalar.activation(ex[:], sc[:], Exp, scale=float(SCALE))
                    else:
                        sc3 = sc[:].rearrange("p (h q) -> p h q", h=2)[:, :, band.start:band.stop]
                        ex3 = ex[:].rearrange("p (h q) -> p h q", h=2)[:, :, band.start:band.stop]
                        nc.scalar.activation(ex3, sc3, Exp, scale=float(SCALE))
                    if d >= 0:
                        mband = slice(128 * d, 128 * (d + 1))
                        exm = ex[:].rearrange("p (h q) -> p h q", h=2)[:, :, mband]
                        mk3 = mk_sb[:].rearrange("p (h q) -> p h q", h=1).broadcast_to((128, 2, 128))
                        nc.vector.tensor_tensor(exm, exm, mk3, MUL)
                    if pend is not None:
                        # ration filler: early chunks take little so the
                        # exp-paced tails of long chunks stay fed
                        if c == 3:
                            emit_filler(2)
                        elif c == 2:
                            emit_filler(1)
                        elif j % 2 == 1:
                            emit_filler(1)
                        emit_ctx(pend)
                    pend = (j, band, ex)
                emit_ctx(pend)
                # normalize first: the den-copy/mult release this chunk's psum
                # bank for the next chunk; filler would delay them in the DVE FIFO
                for h in range(2):
                    rows = slice(64 * h, 64 * (h + 1))
                    den = dv_pool.tile([1, 512], f32, tag="den")
                    nc.vector.tensor_copy(den[:], ps_ctx[h][64:65, :])
                    rec = dv_pool.tile([1, 512], f32, tag="rec")
                    nc.vector.reciprocal_approx_fast(out=rec[:], in_=den[:])
                    bc = dv_pool.tile([64, 512], f32, tag="bc")
                    nc.gpsimd.partition_broadcast(bc[:], rec[:])
                    nc.vector.tensor_tensor(CTX[rows, qcols], ps_ctx[h][0:64, :], bc[:], MUL)
                emit_filler(1)
                # this chunk's output projection becomes PE filler for later chunks
                obholder = {}
                for tb in range(16 * b + 4 * c, 16 * b + 4 * (c + 1)):
                    def ph3_quant(tb=tb, obholder=obholder):
                        grp, idx = divmod(tb, 4)
                        if idx == 0:
                            obholder["ob"] = ob_pool.tile([128, 4, 1024], bf16, tag="ob", name="ob")
                        ob = obholder["ob"]
                        for e in range(2):
                            po = p1.tile([128, 512], f32, tag="p1", name="po")
                            nc.tensor.matmul(
                                po[:],
                                CTX[:, 128 * tb:128 * (tb + 1)],
                                wo_sb[:, 512 * e:512 * (e + 1)],
                                start=True, stop=True,
                            )
                            nc.vector.tensor_copy(ob[:, idx, 512 * e:512 * (e + 1)], po[:])
                        if idx == 3:
                            nc.gpsimd.dma_start(
                                out=outp[128 * grp:128 * (grp + 1), :], in_=ob[:])
                    filler.append(ph3_quant)

            # ---- schedule: loads run one tcpair ahead; projection matmuls and
            # ---- output-projection blocks fill PE gaps inside attention chunks
            for qf in ph1_quanta(0, *tiles0):
                qf()
            chunk_of_tp = {0: (0, 0, 1), 1: (0, 2, 3), 2: (1, 0, 1), 3: (1, 2, 3)}
            for tp in range(1, 4):
                tiles = ph1_loads(tp)
                filler.extend(ph1_quanta(tp, *tiles))
                b, ca, cb = chunk_of_tp[tp - 1]
                ph2_chunk(b, ca)
                ph2_chunk(b, cb)
            b, ca, cb = chunk_of_tp[3]
            ph2_chunk(b, ca)
            ph2_chunk(b, cb)
            emit_filler(len(filler))

    nc.compile()
    return nc


def _host_inputs(q, k, v, mask, w_q, w_k, w_v, w_o):
    import ml_dtypes

    nbf = ml_dtypes.bfloat16

    def arrange_x(x):
        # [T, D] tokens-major -> [slab, piece, partition, featblock*token]
        x2 = np.asarray(x, dtype=np.float32).reshape(T, D).T.astype(nbf)  # [D, T]
        x4 = x2.reshape(8, 128, 4, 2, 512)          # (a, p, tp, half, t)
        return np.ascontiguousarray(x4.transpose(2, 3, 1, 0, 4).reshape(1024, 4096))

    def arrange_w(wT):
        # wT [D, DPC] -> [128, 8*128] partition-major image of the SBUF tile
        w4 = wT.astype(nbf).reshape(8, 128, 128)    # (a, p, d)
        return np.ascontiguousarray(w4.transpose(1, 0, 2).reshape(128, 1024))

    w_q = np.asarray(w_q, dtype=np.float32)
    w_k = np.asarray(w_k, dtype=np.float32)
    w_v = np.asarray(w_v, dtype=np.float32)
    w_o = np.asarray(w_o, dtype=np.float32)
    mask2d = np.asarray(mask).reshape(S, S)

    q4, k4, v4 = arrange_x(q), arrange_x(k), arrange_x(v)
    # single 128x128 tril mask for the mixed band of every diagonal block:
    # valid(r, u) = mask2d[u, r] on the leading 128x128 (= u >= r for causal)
    mk = np.ascontiguousarray(mask2d[0:128, 0:128].T.astype(nbf))

    in_maps = []
    for m in range(N_CORES):
        sl = slice(DPC * m, DPC * (m + 1))
        in_maps.append({
            "qT": q4,
            "kT": k4,
            "vT": v4,
            "wqT": arrange_w(w_q[sl, :].T),
            "wkT": arrange_w(w_k[sl, :].T),
            "wvT": arrange_w(w_v[sl, :].T),
            "woT": np.ascontiguousarray(w_o[:, sl].T.astype(nbf)),
            "msk": mk,
        })
    return in_maps


def kernel(q, k, v, mask, w_q, w_k, w_v, w_o, _trace=False, _results=None):
    in_maps = _host_inputs(q, k, v, mask, w_q, w_k, w_v, w_o)
    if "nc" not in _CACHED:
        _CACHED["nc"] = build_nc()
    nc = _CACHED["nc"]
    res = run_bass_kernel_spmd(
        nc, in_maps, core_ids=list(range(N_CORES)), trace=_trace
    )
    if _results is not None:
        _results.append(res)
    out = np.zeros((1024, 4096), dtype=np.float32)
    for m in range(N_CORES):
        out += np.asarray(res.results[m]["outp"], dtype=np.float32)
    # [group, partition, tokblock*feat] -> [T, D]
    out = out.reshape(8, 128, 4, 1024).transpose(0, 2, 1, 3).reshape(T, D)
    return out.reshape(B, S, D)
